# revision 9
# baseline (speedup 1.0000x reference)
"""Trainium2 Bass kernel for a dense pre-norm transformer block.

Reference computation (fp32):
    h = LN1(x); qkv = h @ qkv_w + qkv_b; attention (16 heads, no 1/sqrt(d));
    x = x + attn_out @ proj_w + proj_b;
    h2 = LN2(x); x = x + gelu_exact(h2 @ fc1_w + fc1_b) @ fc2_w + fc2_b

Shapes: x [2, 2048, 1024], heads 16 x 64, MLP 4096.

Sharding (8 NeuronCores, zero collectives):
    cores 0-3 -> batch 0, cores 4-7 -> batch 1. Each core owns 512 query
    tokens. The host ROTATES the token axis per core (np.roll) so that the
    core's queries are always tokens 0:512 -> one SPMD program for all
    cores. Each core computes LN1 + K + V over all 2048 tokens of its batch
    (cheap replication beats ~100-200us collectives), then Q/attention/
    proj/MLP only for its 512 queries. Attention is permutation-invariant
    over keys, so the rotation does not change results.

Layout: activations are kept feature-major [C, tokens] on-chip.
    - partition-axis reductions (LN stats, softmax sums) via matmul with a
      ones vector; softmax sums ride along the attn@V matmul by appending a
      ones column to V.
    - partition-axis broadcasts (LN mean/rstd, 1/sum) via rank-1 matmuls.
    - all matmul inputs are float32r (TF32-like, 1 cycle/row at N>=256,
      ~1e-4 relative error); PSUM accumulation is fp32.
    - softmax skips the max-subtraction: scores are ~N(0, 3.3^2) given the
      0.02-scaled weights, so exp() stays well inside fp32 range.
"""

import sys

if "/opt/trn_rl_repo" not in sys.path:
    sys.path.insert(0, "/opt/trn_rl_repo")

import numpy as np

import concourse.bass as bass
import concourse.mybir as mybir
import concourse.tile as tile
from concourse import bacc
from concourse.bass_utils import run_bass_kernel_spmd

F32 = mybir.dt.float32
F32R = mybir.dt.float32r
AF = mybir.ActivationFunctionType
ALU = mybir.AluOpType

DIM = 1024
CT = DIM // 128          # 8 feature tiles
NTOK = 2048              # tokens per batch
NQ = 512                 # query tokens per core
H = 16
D = 64
MLP = 4096
FT = MLP // 128          # 32 mlp feature tiles
EPS = 1e-5
N_CORES = 8
GELU_AF = None  # test hook: set to AF.Identity to bypass gelu in CoreSim
NGRP = 4                 # attention processed in 4 groups of 2 head-pairs
GP = 2                   # head pairs per group (256 qkv columns)


def _dma(nc, out, in_):
    nc.gpsimd.dma_start(out=out, in_=in_)


def _col(v):
    """1-D dram AP [n] -> [n, 1] for partition-major bias loads."""
    return v.rearrange("(p o) -> p o", o=1)


def _row(v):
    """1-D dram AP [n] -> [1, n]."""
    return v.rearrange("(o f) -> o f", o=1)


def _ln_stats(nc, sb_stat, mu_ps, musq_ps, ntok_norm):
    """From accumulated sum / sum-of-squares psums [1, NQ] produce
    rstd [1,NQ] f32r and mean*rstd [1,NQ] f32r (sbuf)."""
    mean = sb_stat.tile([1, NQ], F32, tag="mean")
    msq = sb_stat.tile([1, NQ], F32, tag="msq")
    nc.scalar.mul(mean[:], mu_ps[:], 1.0 / ntok_norm)
    nc.scalar.mul(msq[:], musq_ps[:], 1.0 / ntok_norm)
    m2 = sb_stat.tile([1, NQ], F32, tag="m2")
    nc.vector.tensor_mul(m2[:], mean[:], mean[:])
    var = sb_stat.tile([1, NQ], F32, tag="var")
    nc.vector.tensor_sub(var[:], msq[:], m2[:])
    ve = sb_stat.tile([1, NQ], F32, tag="ve")
    nc.vector.tensor_scalar_add(ve[:], var[:], EPS)
    vr = sb_stat.tile([1, NQ], F32, tag="vr")
    nc.vector.reciprocal(vr[:], ve[:])
    rstd = sb_stat.tile([1, NQ], F32R, tag="rstd", bufs=4, name="rstd")
    nc.scalar.activation(rstd[:], vr[:], AF.Sqrt)
    mrs = sb_stat.tile([1, NQ], F32R, tag="mrs", bufs=4, name="mrs")
    nc.vector.tensor_mul(mrs[:], mean[:], rstd[:])
    return rstd, mrs


def build_program():
    nc = bacc.Bacc("TRN2", target_bir_lowering=False)

    xT = nc.declare_dram_parameter("xT", [DIM, NTOK], F32, isOutput=False)
    qkv_w = nc.declare_dram_parameter("qkv_w", [DIM, 3 * DIM], F32, isOutput=False)
    qkv_b = nc.declare_dram_parameter("qkv_b", [3 * DIM], F32, isOutput=False)
    proj_w = nc.declare_dram_parameter("proj_w", [DIM, DIM], F32, isOutput=False)
    proj_b = nc.declare_dram_parameter("proj_b", [DIM], F32, isOutput=False)
    ln1_g = nc.declare_dram_parameter("ln1_g", [DIM], F32, isOutput=False)
    ln1_b = nc.declare_dram_parameter("ln1_b", [DIM], F32, isOutput=False)
    ln2_g = nc.declare_dram_parameter("ln2_g", [DIM], F32, isOutput=False)
    ln2_b = nc.declare_dram_parameter("ln2_b", [DIM], F32, isOutput=False)
    fc1_w = nc.declare_dram_parameter("fc1_w", [DIM, MLP], F32, isOutput=False)
    fc1_b = nc.declare_dram_parameter("fc1_b", [MLP], F32, isOutput=False)
    fc2_w = nc.declare_dram_parameter("fc2_w", [MLP, DIM], F32, isOutput=False)
    fc2_b = nc.declare_dram_parameter("fc2_b", [DIM], F32, isOutput=False)
    ones_in = nc.declare_dram_parameter("ones_in", [128], F32, isOutput=False)
    outT = nc.declare_dram_parameter("outT", [DIM, NQ], F32, isOutput=True)

    with tile.TileContext(nc) as tc:
        with (
            tc.tile_pool(name="const", bufs=1) as const,
            tc.tile_pool(name="xres", bufs=CT) as xres_pool,
            tc.tile_pool(name="yT", bufs=CT) as yT_pool,
            tc.tile_pool(name="stat", bufs=1) as sb_stat,
        ):
            # ---- constants ----
            ones_col = const.tile([128, 1], F32R, tag="ones_col")
            _dma(nc, ones_col[:], _col(ones_in[:]))
            ones_row = const.tile([1, 128], F32R, tag="ones_row")
            _dma(nc, ones_row[:], _row(ones_in[:]))
            ones8 = const.tile([128, 8], F32R, tag="ones8")
            for i in range(8):
                _dma(nc, ones8[:, i : i + 1], _col(ones_in[:]))

            ln1g_t = const.tile([128, CT], F32, tag="ln1g")
            ln1b_t = const.tile([128, CT], F32, tag="ln1b")
            ln2g_t = const.tile([128, CT], F32, tag="ln2g")
            ln2b_t = const.tile([128, CT], F32, tag="ln2b")
            projb_t = const.tile([128, CT], F32, tag="projb")
            fc2b_t = const.tile([128, CT], F32, tag="fc2b")
            fc1b_t = const.tile([128, FT], F32, tag="fc1b")
            qb_q = const.tile([128, CT], F32, tag="qbq")
            qb_k = const.tile([128, CT], F32, tag="qbk")
            vb = const.tile([1, DIM], F32R, tag="vb")
            for ct in range(CT):
                sl = slice(ct * 128, (ct + 1) * 128)
                _dma(nc, ln1g_t[:, ct : ct + 1], _col(ln1_g[sl]))
                _dma(nc, ln1b_t[:, ct : ct + 1], _col(ln1_b[sl]))
                _dma(nc, ln2g_t[:, ct : ct + 1], _col(ln2_g[sl]))
                _dma(nc, ln2b_t[:, ct : ct + 1], _col(ln2_b[sl]))
                _dma(nc, projb_t[:, ct : ct + 1], _col(proj_b[sl]))
                _dma(nc, fc2b_t[:, ct : ct + 1], _col(fc2_b[sl]))
                _dma(nc, qb_q[:, ct : ct + 1], _col(qkv_b[sl]))
                _dma(nc, qb_k[:, ct : ct + 1],
                     _col(qkv_b[DIM + ct * 128 : DIM + (ct + 1) * 128]))
            for ft in range(FT):
                _dma(nc, fc1b_t[:, ft : ft + 1], _col(fc1_b[ft * 128 : (ft + 1) * 128]))
            _dma(nc, vb[:], _row(qkv_b[2 * DIM : 3 * DIM]))

            xres = [xres_pool.tile([128, NQ], F32R, tag="xres", name=f"xres{i}") for i in range(CT)]
            yT = [yT_pool.tile([128, NQ], F32R, tag="yT", name=f"yT{i}") for i in range(CT)]

            with tc.tile_pool(name="h1p", bufs=CT) as h1_pool:
                h1 = [h1_pool.tile([128, NTOK], F32R, tag="h1", name=f"h1_{i}") for i in range(CT)]

                # ================= stage A: LN1 over all 2048 tokens ========
                # pass 1: stream x, accumulate per-chunk sum / sum-of-squares
                stats = []
                with (
                    tc.tile_pool(name="xp1", bufs=2) as x_pool1,
                    tc.tile_pool(name="sqp", bufs=2) as sq_pool,
                    tc.tile_pool(name="psA", bufs=4, space="PSUM") as psA,
                ):
                    mu_ps = [psA.tile([1, NQ], F32, tag="mu", name=f"mu{i}")
                             for i in range(4)]
                    musq_ps = [psA.tile([1, NQ], F32, tag="musq", name=f"musq{i}")
                               for i in range(4)]
                    for ct in range(CT):
                        xs = x_pool1.tile([128, NTOK], F32R, tag="xs", name="xs")
                        _dma(nc, xs[:], xT[ct * 128 : (ct + 1) * 128, :])
                        sq = sq_pool.tile([128, NTOK], F32R, tag="sq", name="sq")
                        nc.vector.tensor_mul(sq[:], xs[:], xs[:])
                        for ch in range(4):
                            csl = slice(ch * NQ, (ch + 1) * NQ)
                            nc.tensor.matmul(
                                mu_ps[ch][:], ones_col[:], xs[:, csl],
                                start=(ct == 0), stop=(ct == CT - 1))
                            nc.tensor.matmul(
                                musq_ps[ch][:], ones_col[:], sq[:, csl],
                                start=(ct == 0), stop=(ct == CT - 1))
                    for ch in range(4):
                        stats.append(_ln_stats(nc, sb_stat, mu_ps[ch], musq_ps[ch], DIM))
                # pass 2: broadcast stats, re-stream x, normalize into h1
                with (
                    tc.tile_pool(name="xp2", bufs=2) as x_pool2,
                    tc.tile_pool(name="lnw", bufs=2) as ln_work,
                    tc.tile_pool(name="psAb", bufs=4, space="PSUM") as psAb,
                ):
                    bc_rstd = []
                    bc_mrs = []
                    for ch in range(4):
                        rstd, mrs = stats[ch]
                        bcr = psAb.tile([128, NQ], F32, tag="bcr", name=f"bcr{ch}")
                        bcm = psAb.tile([128, NQ], F32, tag="bcm", name=f"bcm{ch}")
                        nc.tensor.matmul(bcr[:], ones_row[:], rstd[:],
                                         start=True, stop=True)
                        nc.tensor.matmul(bcm[:], ones_row[:], mrs[:],
                                         start=True, stop=True)
                        bc_rstd.append(bcr)
                        bc_mrs.append(bcm)
                    for ct in range(CT):
                        xs = x_pool2.tile([128, NTOK], F32R, tag="xs2", name="xs2")
                        _dma(nc, xs[:], xT[ct * 128 : (ct + 1) * 128, :])
                        nc.vector.tensor_copy(xres[ct][:], xs[:, 0:NQ])
                        for ch in range(4):
                            csl = slice(ch * NQ, (ch + 1) * NQ)
                            t = ln_work.tile([128, NQ], F32, tag="lnt", name="lnt")
                            nc.vector.tensor_mul(t[:], xs[:, csl], bc_rstd[ch][:])
                            z = ln_work.tile([128, NQ], F32, tag="lnz", name="lnz")
                            nc.vector.tensor_sub(z[:], t[:], bc_mrs[ch][:])
                            nc.scalar.activation(
                                h1[ct][:, csl], z[:], AF.Identity,
                                bias=ln1b_t[:, ct : ct + 1],
                                scale=ln1g_t[:, ct : ct + 1])

                # ====== stages B+C: QKV + attention, 4 groups of 2 pairs ====
                for grp in range(NGRP):
                    gw = GP * 128  # 256 qkv columns per group
                    g0 = grp * gw
                    with tc.tile_pool(name="kvp", bufs=1) as kv_pool:
                        kT = [kv_pool.tile([128, NTOK], F32R, tag=f"kT{p4}", name=f"kT{p4}")
                              for p4 in range(GP)]
                        qT = [kv_pool.tile([128, NQ], F32R, tag=f"qT{p4}", name=f"qT{p4}")
                              for p4 in range(GP)]
                        vaug = [kv_pool.tile([128, 2 * GP, 65], F32R, tag=f"va{t}", name=f"va{t}")
                                for t in range(16)]

                        with tc.tile_pool(name="psB", bufs=3, space="PSUM") as psB:
                            # K path: feature-major [128 pair-dims, 2048 keys]
                            with tc.tile_pool(name="wkp", bufs=CT) as wk_pool:
                                wk_t = [wk_pool.tile([128, gw], F32R, tag="wk", name=f"wk{i}") for i in range(CT)]
                                for ct in range(CT):
                                    rsl = slice(ct * 128, (ct + 1) * 128)
                                    _dma(nc, wk_t[ct][:], qkv_w[rsl, DIM + g0 : DIM + g0 + gw])
                                for p4 in range(GP):
                                    p = grp * GP + p4
                                    psl = slice(p4 * 128, (p4 + 1) * 128)
                                    for ch in range(4):
                                        csl = slice(ch * NQ, (ch + 1) * NQ)
                                        ps = psB.tile([128, NQ], F32, tag="bps", name="bps")
                                        for ct in range(CT):
                                            nc.tensor.matmul(
                                                ps[:], wk_t[ct][:, psl], h1[ct][:, csl],
                                                start=(ct == 0), stop=(ct == CT - 1))
                                        nc.scalar.activation(
                                            kT[p4][:, csl], ps[:], AF.Identity,
                                            bias=qb_k[:, p : p + 1])
                            # Q path (queries = tokens 0:512)
                            with tc.tile_pool(name="wqp", bufs=CT) as wq_pool:
                                wq_t = [wq_pool.tile([128, gw], F32R, tag="wq", name=f"wq{i}") for i in range(CT)]
                                for ct in range(CT):
                                    rsl = slice(ct * 128, (ct + 1) * 128)
                                    _dma(nc, wq_t[ct][:], qkv_w[rsl, g0 : g0 + gw])
                                for p4 in range(GP):
                                    p = grp * GP + p4
                                    psl = slice(p4 * 128, (p4 + 1) * 128)
                                    ps = psB.tile([128, NQ], F32, tag="bps", name="bps")
                                    for ct in range(CT):
                                        nc.tensor.matmul(
                                            ps[:], wq_t[ct][:, psl], h1[ct][:, 0:NQ],
                                            start=(ct == 0), stop=(ct == CT - 1))
                                    nc.scalar.activation(
                                        qT[p4][:], ps[:], AF.Identity,
                                        bias=qb_q[:, p : p + 1])
                            # V path: token-major [128 tokens, 256 vdims] + ones
                            with tc.tile_pool(name="wvp", bufs=CT) as wv_pool:
                                wv_t = [wv_pool.tile([128, gw], F32R, tag="wv", name=f"wv{i}") for i in range(CT)]
                                for ct in range(CT):
                                    rsl = slice(ct * 128, (ct + 1) * 128)
                                    _dma(nc, wv_t[ct][:], qkv_w[rsl, 2 * DIM + g0 : 2 * DIM + g0 + gw])
                                for tt in range(16):
                                    tsl = slice(tt * 128, (tt + 1) * 128)
                                    ps = psB.tile([128, gw], F32, tag="vps", name="vps")
                                    for ct in range(CT):
                                        nc.tensor.matmul(
                                            ps[:], h1[ct][:, tsl], wv_t[ct][:],
                                            start=(ct == 0), stop=False)
                                    nc.tensor.matmul(
                                        ps[:], ones_row[:], vb[:, g0 : g0 + gw],
                                        start=False, stop=True)
                                    nc.scalar.copy(
                                        vaug[tt][:, :, 0:64],
                                        ps[:].rearrange("p (a f) -> p a f", f=64))
                                    nc.vector.tensor_copy(
                                        vaug[tt][:, :, 64:65],
                                        ones8[:, 0 : 2 * GP].rearrange("p (a o) -> p a o", o=1))

                        # ---- stage C: attention for this group's 4 heads ----
                        with (
                            tc.tile_pool(name="ep", bufs=3) as e_pool,
                            tc.tile_pool(name="ytp", bufs=2) as yt_pool,
                            tc.tile_pool(name="psS", bufs=3, space="PSUM") as psS,
                            tc.tile_pool(name="psAV", bufs=2, space="PSUM") as psAV,
                            tc.tile_pool(name="psRB", bufs=2, space="PSUM") as psRB,
                        ):
                            for p4 in range(GP):
                                p = grp * GP + p4
                                for hh in range(2):
                                    hsl = slice(hh * 64, (hh + 1) * 64)
                                    av = psAV.tile([65, NQ], F32, tag="av")
                                    for kt in range(16):
                                        s_ps = psS.tile([128, NQ], F32, tag="s")
                                        nc.tensor.matmul(
                                            s_ps[:],
                                            kT[p4][hsl, kt * 128 : (kt + 1) * 128],
                                            qT[p4][hsl, :],
                                            start=True, stop=True)
                                        e_t = e_pool.tile([128, NQ], F32R, tag="e")
                                        nc.scalar.activation(e_t[:], s_ps[:], AF.Exp)
                                        nc.tensor.matmul(
                                            av[:], vaug[kt][:, p4 * 2 + hh, :], e_t[:],
                                            start=(kt == 0), stop=(kt == 15))
                                    rcp = sb_stat.tile([1, NQ], F32R, tag="rcp")
                                    with nc.allow_low_precision("softmax 1/sum in f32r"):
                                        nc.vector.reciprocal(rcp[:], av[64:65, :])
                                    rb = psRB.tile([64, NQ], F32, tag="rb")
                                    nc.tensor.matmul(rb[:], ones_row[:, 0:64], rcp[:],
                                                     start=True, stop=True)
                                    ytmp = yt_pool.tile([64, NQ], F32, tag="ytmp")
                                    nc.scalar.copy(ytmp[:], av[0:64, :])
                                    nc.vector.tensor_mul(yT[p][hsl, :], ytmp[:], rb[:])

            # ================= stage D: proj + residual =====================
            with tc.tile_pool(name="x2p", bufs=CT) as x2_pool:
                x2 = [x2_pool.tile([128, NQ], F32R, tag="x2", name=f"x2_{i}") for i in range(CT)]
                with (
                    tc.tile_pool(name="pwp", bufs=CT) as pw_pool,
                    tc.tile_pool(name="psD", bufs=2, space="PSUM") as psD,
                ):
                    pw_t = [pw_pool.tile([128, DIM], F32R, tag="pw", name=f"pw{i}") for i in range(CT)]
                    for ct in range(CT):
                        _dma(nc, pw_t[ct][:], proj_w[ct * 128 : (ct + 1) * 128, :])
                    for co in range(CT):
                        ps = psD.tile([128, NQ], F32, tag="dps")
                        for ct in range(CT):
                            nc.tensor.matmul(
                                ps[:], pw_t[ct][:, co * 128 : (co + 1) * 128], yT[ct][:],
                                start=(ct == 0), stop=(ct == CT - 1))
                        nc.vector.scalar_tensor_tensor(
                            x2[co][:], ps[:], projb_t[:, co : co + 1], xres[co][:],
                            op0=ALU.add, op1=ALU.add)

                # ============= stage E: LN2 (512 tokens) ====================
                with tc.tile_pool(name="h2p", bufs=CT) as h2_pool:
                    h2 = [h2_pool.tile([128, NQ], F32R, tag="h2", name=f"h2_{i}") for i in range(CT)]
                    with (
                        tc.tile_pool(name="psE", bufs=1, space="PSUM") as psE,
                        tc.tile_pool(name="psEb", bufs=1, space="PSUM") as psEb,
                        tc.tile_pool(name="sq2p", bufs=2) as sq2_pool,
                        tc.tile_pool(name="lnw2", bufs=2) as ln_work2,
                    ):
                        mu_ps = psE.tile([1, NQ], F32, tag="mu2")
                        musq_ps = psE.tile([1, NQ], F32, tag="musq2")
                        for ct in range(CT):
                            sq = sq2_pool.tile([128, NQ], F32R, tag="sq2")
                            nc.vector.tensor_mul(sq[:], x2[ct][:], x2[ct][:])
                            nc.tensor.matmul(mu_ps[:], ones_col[:], x2[ct][:],
                                             start=(ct == 0), stop=(ct == CT - 1))
                            nc.tensor.matmul(musq_ps[:], ones_col[:], sq[:],
                                             start=(ct == 0), stop=(ct == CT - 1))
                        rstd, mrs = _ln_stats(nc, sb_stat, mu_ps, musq_ps, DIM)
                        bc_rstd = psEb.tile([128, NQ], F32, tag="bcr2")
                        bc_mrs = psEb.tile([128, NQ], F32, tag="bcm2")
                        nc.tensor.matmul(bc_rstd[:], ones_row[:], rstd[:],
                                         start=True, stop=True)
                        nc.tensor.matmul(bc_mrs[:], ones_row[:], mrs[:],
                                         start=True, stop=True)
                        for ct in range(CT):
                            t = ln_work2.tile([128, NQ], F32, tag="lnt2")
                            nc.vector.tensor_mul(t[:], x2[ct][:], bc_rstd[:])
                            z = ln_work2.tile([128, NQ], F32, tag="lnz2")
                            nc.vector.tensor_sub(z[:], t[:], bc_mrs[:])
                            nc.scalar.activation(
                                h2[ct][:], z[:], AF.Identity,
                                bias=ln2b_t[:, ct : ct + 1],
                                scale=ln2g_t[:, ct : ct + 1])

                    # ============ stage F: MLP ==============================
                    with tc.tile_pool(name="gp", bufs=FT) as g_pool:
                        g_t = [g_pool.tile([128, NQ], F32R, tag="g", name=f"g{i}") for i in range(FT)]
                        # fc1 + exact gelu: fo groups of 4 psum banks
                        with (
                            tc.tile_pool(name="w1p", bufs=12) as w1_pool,
                            tc.tile_pool(name="psF1", bufs=8, space="PSUM") as psF1,
                        ):
                            for fog in range(8):
                                w1_t = [w1_pool.tile([128, 512], F32R, tag="w1", name=f"w1_{i}")
                                        for i in range(CT)]
                                for ct in range(CT):
                                    _dma(nc, w1_t[ct][:],
                                         fc1_w[ct * 128 : (ct + 1) * 128,
                                               fog * 512 : (fog + 1) * 512])
                                pss = [psF1.tile([128, NQ], F32, tag="f1ps", name=f"f1ps{i}")
                                       for i in range(4)]
                                for ct in range(CT):
                                    for fo4 in range(4):
                                        nc.tensor.matmul(
                                            pss[fo4][:],
                                            w1_t[ct][:, fo4 * 128 : (fo4 + 1) * 128],
                                            h2[ct][:],
                                            start=(ct == 0), stop=(ct == CT - 1))
                                for fo4 in range(4):
                                    fo = fog * 4 + fo4
                                    nc.scalar.activation(
                                        g_t[fo][:], pss[fo4][:],
                                        GELU_AF or AF.Gelu,
                                        bias=fc1b_t[:, fo : fo + 1])
                        # fc2 + residual
                        with (
                            tc.tile_pool(name="w2p", bufs=4) as w2_pool,
                            tc.tile_pool(name="psF2", bufs=8, space="PSUM") as psF2,
                            tc.tile_pool(name="op", bufs=4) as out_pool,
                        ):
                            for cog in range(2):
                                pss = [psF2.tile([128, NQ], F32, tag="f2ps", name=f"f2ps{i}")
                                       for i in range(4)]
                                for ko in range(FT):
                                    w2_t = w2_pool.tile([128, 512], F32R, tag="w2")
                                    _dma(nc, w2_t[:],
                                         fc2_w[ko * 128 : (ko + 1) * 128,
                                               cog * 512 : (cog + 1) * 512])
                                    for co4 in range(4):
                                        nc.tensor.matmul(
                                            pss[co4][:],
                                            w2_t[:, co4 * 128 : (co4 + 1) * 128],
                                            g_t[ko][:],
                                            start=(ko == 0), stop=(ko == FT - 1))
                                for co4 in range(4):
                                    co = cog * 4 + co4
                                    o_t = out_pool.tile([128, NQ], F32, tag="o")
                                    nc.vector.scalar_tensor_tensor(
                                        o_t[:], pss[co4][:], fc2b_t[:, co : co + 1],
                                        x2[co][:], op0=ALU.add, op1=ALU.add)
                                    _dma(nc, outT[co * 128 : (co + 1) * 128, :], o_t[:])

    nc.compile()
    return nc


_CACHED_NC = None


def _get_nc():
    global _CACHED_NC
    if _CACHED_NC is None:
        _CACHED_NC = build_program()
    return _CACHED_NC


def make_in_maps(inputs):
    ins = {k: np.ascontiguousarray(np.asarray(v), dtype=np.float32)
           for k, v in inputs.items()}
    in_maps = []
    for core in range(N_CORES):
        b = core // 4
        qs = (core % 4) * NQ
        x_rot = np.roll(ins["x"][b], -qs, axis=0)
        in_maps.append({
            "xT": np.ascontiguousarray(x_rot.T),
            "ones_in": np.ones(128, np.float32),
            "qkv_w": ins["qkv_w"], "qkv_b": ins["qkv_b"],
            "proj_w": ins["proj_w"], "proj_b": ins["proj_b"],
            "ln1_g": ins["ln1_g"], "ln1_b": ins["ln1_b"],
            "ln2_g": ins["ln2_g"], "ln2_b": ins["ln2_b"],
            "fc1_w": ins["fc1_w"], "fc1_b": ins["fc1_b"],
            "fc2_w": ins["fc2_w"], "fc2_b": ins["fc2_b"],
        })
    return in_maps


def gather_output(results):
    out = np.empty((2, NTOK, DIM), dtype=np.float32)
    for core in range(N_CORES):
        b = core // 4
        qs = (core % 4) * NQ
        out[b, qs : qs + NQ, :] = results[core]["outT"].T
    return out


def kernel(**inputs) -> np.ndarray:
    nc = _get_nc()
    in_maps = make_in_maps(inputs)
    res = run_bass_kernel_spmd(nc, in_maps, list(range(N_CORES)))
    return gather_output(res.results)


if __name__ == "__main__":
    rng = np.random.default_rng(0)
    demo = {
        "x": rng.standard_normal((2, NTOK, DIM), dtype=np.float32),
        "ln1_g": np.ones(DIM, np.float32), "ln1_b": np.zeros(DIM, np.float32),
        "qkv_w": (rng.standard_normal((DIM, 3 * DIM)) * 0.02).astype(np.float32),
        "qkv_b": np.zeros(3 * DIM, np.float32),
        "proj_w": (rng.standard_normal((DIM, DIM)) * 0.02).astype(np.float32),
        "proj_b": np.zeros(DIM, np.float32),
        "ln2_g": np.ones(DIM, np.float32), "ln2_b": np.zeros(DIM, np.float32),
        "fc1_w": (rng.standard_normal((DIM, MLP)) * 0.02).astype(np.float32),
        "fc1_b": np.zeros(MLP, np.float32),
        "fc2_w": (rng.standard_normal((MLP, DIM)) * 0.02).astype(np.float32),
        "fc2_b": np.zeros(DIM, np.float32),
    }
    out = kernel(**demo)
    print("out", out.shape, out.dtype, float(np.abs(out).max()))
